# revision 43
# baseline (speedup 1.0000x reference)
"""Trainium2 Bass kernel for nn_BlankEmbedding (embedding gather + blank-run scan).

Math: the reference computes e = emb_table[x] then 8 shift/accumulate
iterations seeded at pre-blank positions.  Unrolled, out[i] =
sum_{d=0..8} C[i,d] * e[i-d] with banded integer coefficients C that
depend only on x.  Rows with any C[i,d>0] != 0 ("affected") are sparse
(~6% at the reference blank density); every other row is out[i] = e[i].

Kernel design (per core: 2048 of the 16384 [B*S] rows, data-parallel):
  - the table is cast to fp16 on host (values ~N(0, 0.02); fp16 rounding
    is ~5e-4 relative vs the 2e-2 gate), halving all device traffic; the
    host upcasts the assembled output back to fp32.
  - 16 aligned tiles of 128 rows: ONE hardware-indirect DMA gathers the
    tile's embedding rows ([128,1] int32 offset column read by the DGE -
    no per-row ucode), and a dense writeback DMA streams it straight
    back out SBUF -> DRAM.  This pure gather->writeback pipeline is
    DMA-bound; TensorE/DVE do nothing for the 94% identity rows.
  - affected rows are packed into NG compact groups (<=128 targets and
    <=128 deduped band-source rows each).  Per group: one indirect
    gather of the sources (issued FIRST so it never waits behind the
    main stream), 4 matmuls (fp16 x fp16 -> fp32 PSUM, 512-col chunks =
    one PSUM bank) with a host-built [src,tgt] coefficient matrix that
    includes the d=0 term, a DVE copy PSUM -> fp16 SBUF, and one
    indirect-scatter DMA that OVERWRITES the affected output rows after
    every direct writeback has completed.
  - padding: dead source slots point at an all-zero table row (row NV);
    dead target slots scatter into a dummy output row (row RPC).

Sync notes (hard-won): a DMA's +16 semaphore update arrives as 16
per-engine +1s, so every gather/writeback that gates something gets its
own semaphore; and engine program order does not cover a compute op's
SBUF write drain before a same-engine dma_start, so DMA-after-compute
is always gated through a semaphore (here: copies on DVE, scatters on
GPSIMD).

All differences between cores are input data (indices, coefficients),
so one program per group-count NG is compiled and reused.
"""

import numpy as np

B, S, D = 4, 4096, 2048
N_CORES = 8
RPC = (B * S) // N_CORES   # rows per core = 2048
NV = 2048                  # x < 2048 per the problem spec -> table slice
NTM = 16                   # main tiles of 128 rows
TSPLIT = 12                # scatter gate split: tiles [0,12) vs [12,16)
RSPLIT = TSPLIT * 128      # groups never span this target-row boundary
K = 128
NB = 9                     # band width: out[i] depends on e[i-8..i]
CH = 512                   # matmul N-chunk = one PSUM bank of fp32
NCH = D // CH
N_ITER = 8


def _compute_coeffs(x):
    """C[b, s, d] for d=0..8 (float64 holds small ints exactly)."""
    b, s = x.shape
    blank = ((x >= 0) & (x < 16)).astype(np.float64)
    shift_r = lambda t: np.concatenate([np.zeros_like(t[:, :1]), t[:, :-1]], axis=1)
    first = np.maximum(blank - shift_r(blank), 0.0)
    m = np.concatenate([first[:, 1:], np.zeros_like(first[:, :1])], axis=1)
    C = np.zeros((b, s, NB))
    C[:, :, 0] = 1.0
    for k in range(1, N_ITER + 1):
        m_k = np.zeros_like(m)
        m_k[:, k:] = m[:, :-k]
        Cs = np.zeros_like(C)
        Cs[:, 1:, 1:] = C[:, :-1, :-1]
        C = C + m_k[:, :, None] * Cs
    return C


def _prepare(x_np):
    """Per-core gather indices, scatter targets, group coefficient mats."""
    if x_np.max() < NV and x_np.min() >= 0:
        ridx = x_np.astype(np.int64)
        uniq = None
    else:  # fallback: remap through unique rows (stays within NV slots)
        uniq, inv = np.unique(x_np, return_inverse=True)
        assert len(uniq) <= NV
        ridx = inv.reshape(x_np.shape).astype(np.int64)

    C = _compute_coeffs(x_np)
    cores = []
    for c in range(N_CORES):
        b, h = c // 2, c % 2
        s0 = h * RPC
        # greedy pack affected targets into groups of <=128 targets with
        # <=128 distinct source positions; a group never spans the RSPLIT
        # boundary so each scatter can gate on only its half's writebacks
        glo, ghi = [], []     # region group lists: (srcs, [(tgt, {d: coef})])
        cur_s, cur_t = [], []
        for r in range(RPC):
            if not (C[b, s0 + r, 1:] != 0).any():
                continue
            coefs = {int(d): C[b, s0 + r, d] for d in range(NB)
                     if C[b, s0 + r, d] != 0}
            srcs = [s0 + r - d for d in coefs]
            new = [p for p in srcs if p not in cur_s]
            cross = cur_t and (cur_t[0][0] < RSPLIT) != (r < RSPLIT)
            if cross or len(cur_s) + len(new) > K or len(cur_t) + 1 > K:
                (glo if cur_t[0][0] < RSPLIT else ghi).append((cur_s, cur_t))
                cur_s, cur_t = list(dict.fromkeys(srcs)), [(r, coefs)]
            else:
                cur_s += new
                cur_t.append((r, coefs))
        if cur_t:
            (glo if cur_t[0][0] < RSPLIT else ghi).append((cur_s, cur_t))
        cores.append(dict(b=b, s0=s0, glo=glo, ghi=ghi))

    # uniform slot layout: region-0 group slots first, then region-1
    N0 = max(len(co["glo"]) for co in cores)
    N1 = max(len(co["ghi"]) for co in cores)
    NG = N0 + N1
    REGS = (0,) * N0 + (1,) * N1

    def slot_group(co, g):
        lst, gi = (co["glo"], g) if g < N0 else (co["ghi"], g - N0)
        return lst[gi] if gi < len(lst) else None

    # exact per-group capacities (max over cores; floor 2 because
    # single-element indirect DMAs are unsupported)
    KS, MS = [], []
    for g in range(NG):
        gs = [slot_group(co, g) for co in cores]
        KS.append(max(2, max(len(gr[0]) if gr else 1 for gr in gs)))
        MS.append(max(2, max(len(gr[1]) if gr else 1 for gr in gs)))
    for co in cores:
        b, s0 = co["b"], co["s0"]
        # one extra column: tile 15 is gathered/written as two 64-row
        # halves to halve the end-of-stream drain
        idx = np.zeros((K, NTM + NG + 1), np.int32)
        for t in range(NTM):
            idx[:, NG + t] = ridx[b, s0 + t * K: s0 + (t + 1) * K]
        idx[0:64, NG + NTM] = ridx[b, s0 + 15 * K + 64: s0 + 16 * K]
        tidx = np.full((K, NG), RPC, np.int32)        # pad -> dummy row
        dmat = np.zeros((K, NG * K), np.float16)
        for g in range(NG):
            idx[:, g] = NV                            # pad -> zero row
            gr = slot_group(co, g)
            if gr is not None:
                srcs, tgts = gr
                spos = {p: k for k, p in enumerate(srcs)}
                idx[:len(srcs), g] = [ridx[b, p] for p in srcs]
                for m_i, (r, coefs) in enumerate(tgts):
                    tidx[m_i, g] = r
                    for d, cf in coefs.items():
                        dmat[spos[s0 + r - d], g * K + m_i] = cf
        co.update(idx=idx, tidx=tidx, dmat=dmat)
    return uniq, cores, (NG, REGS, tuple(KS), tuple(MS))


def _build_program(key):
    import concourse.bacc as bacc
    import concourse.mybir as mybir
    from concourse.bass import IndirectOffsetOnAxis

    NG, REGS, KS, MS = key
    f16, f32, i32 = mybir.dt.float16, mybir.dt.float32, mybir.dt.int32
    NTI = NTM + NG + 1        # + tile-15 second half (drain split)
    NW_LO = TSPLIT            # writebacks gating region-0 scatters
    NW_HI = NTM - TSPLIT + 1  # high tiles incl. the tile-15 extra half
    nc = bacc.Bacc("TRN2", target_bir_lowering=False, debug=False,
                   enable_asserts=False, num_devices=N_CORES)
    table_d = nc.dram_tensor("table", [NV + 1, D], f16, kind="ExternalInput")
    idx_d = nc.dram_tensor("idx", [K, NTI], i32, kind="ExternalInput")
    tidx_d = nc.dram_tensor("tidx", [K, NG], i32, kind="ExternalInput")
    dmat_d = nc.dram_tensor("dmat", [K, NG * K], f16, kind="ExternalInput")
    out_d = nc.dram_tensor("out", [RPC + 1, D], f16, kind="ExternalOutput")

    from contextlib import ExitStack
    with ExitStack() as st:
        gtile = st.enter_context(nc.sbuf_tensor("gtile", [K, NTI, D], f16))
        csc = st.enter_context(nc.sbuf_tensor("csc", [K, NG, D], f16))
        dmat_s = st.enter_context(nc.sbuf_tensor("dmat_s", [K, NG * K], f16))
        idx_s = st.enter_context(nc.sbuf_tensor("idx_s", [K, NTI], i32))
        tidx_s = st.enter_context(nc.sbuf_tensor("tidx_s", [K, NG], i32))
        pb = st.enter_context(nc.psum_tensor("pb", [K, 2, D], f32))
        ix_sem = st.enter_context(nc.semaphore("ix_sem"))
        dm_sem = st.enter_context(nc.semaphore("dm_sem"))
        g_sems = [st.enter_context(nc.semaphore(f"g_sem{t}")) for t in range(NTI)]
        t_sem = st.enter_context(nc.semaphore("t_sem"))
        cv_sem = st.enter_context(nc.semaphore("cv_sem"))
        w_lo = st.enter_context(nc.semaphore("w_lo"))
        w_hi = st.enter_context(nc.semaphore("w_hi"))
        s_sem = st.enter_context(nc.semaphore("s_sem"))
        block = st.enter_context(nc.Block(no_gpsimd_drain=True))

        def writeback(eng, t):
            # tile 15 is written as two 64-row halves on DIFFERENT rings
            # (half A here on Act, half B on sync) so the stream drain runs
            # both HWDGE rings in parallel
            ws = w_lo if t < TSPLIT else w_hi
            n = 64 if t == NTM - 1 else K
            eng.wait_ge(g_sems[NG + t], 16)
            eng.dma_start(out_d[t * K:t * K + n, :],
                          gtile[0:n, NG + t, :]).then_inc(ws, 16)

        @block.sync
        def _(sy):
            sy.dma_start(idx_s[:, :], idx_d[:, :]).then_inc(ix_sem, 16)
            for t in range(0, NTM, 2):
                writeback(sy, t)
            sy.wait_ge(g_sems[NG + NTM], 16)   # tile-15 half B
            sy.dma_start(out_d[NTM * K - 64:NTM * K, :],
                         gtile[0:64, NG + NTM, :]).then_inc(w_hi, 16)
            sy.wait_ge(w_lo, 16 * NW_LO)
            sy.wait_ge(w_hi, 16 * NW_HI)
            sy.wait_ge(s_sem, 16 * NG)

        @block.scalar
        def _(sc):
            sc.dma_start(dmat_s[:, :], dmat_d[:, :]).then_inc(dm_sem, 16)
            sc.dma_start(tidx_s[:, :], tidx_d[:, :]).then_inc(dm_sem, 16)
            for t in range(1, NTM, 2):
                writeback(sc, t)

        # first two tile gathers lead (earliest writeback start), then the
        # group sources, then the rest; tile 15's halves close the stream
        issue = ([NG, NG + 1] + list(range(NG))
                 + list(range(NG + 2, NG + NTM)) + [NG + NTM])

        @block.gpsimd
        def _(gp):
            gp.wait_ge(ix_sem, 16)
            for t in issue:
                if t < NG:
                    kk = KS[t]
                elif t == NG + NTM - 1 or t == NG + NTM:
                    kk = 64               # tile-15 halves
                else:
                    kk = K
                gp.indirect_dma_start(
                    out=gtile[0:kk, t, :], out_offset=None,
                    in_=table_d[:, :],
                    in_offset=IndirectOffsetOnAxis(ap=idx_s[0:kk, t:t + 1], axis=0),
                ).then_inc(g_sems[t], 16)
            gp.wait_ge(dm_sem, 32)         # tidx loaded
            for g in range(NG):
                # only the half holding this group's targets must be written;
                # region-0 scatters fire while the high tiles still stream
                if REGS[g] == 0:
                    gp.wait_ge(w_lo, 16 * NW_LO)
                else:
                    gp.wait_ge(w_hi, 16 * NW_HI)
                gp.wait_ge(cv_sem, g + 1)  # corrected rows staged in csc
                gp.indirect_dma_start(
                    out=out_d[:, :],
                    out_offset=IndirectOffsetOnAxis(ap=tidx_s[0:MS[g], g:g + 1],
                                                    axis=0),
                    in_=csc[0:MS[g], g, :], in_offset=None,
                ).then_inc(s_sem, 16)

        @block.tensor
        def _(te):
            te.wait_ge(dm_sem, 32)
            for g in range(NG):
                te.wait_ge(g_sems[g], 16)
                if g >= 2:
                    te.wait_ge(cv_sem, g - 1)  # PSUM slot g%2 free again
                for j in range(NCH):
                    ins = te.matmul(pb[0:MS[g], g % 2, j * CH:(j + 1) * CH],
                                    dmat_s[0:KS[g], g * K:g * K + MS[g]],
                                    gtile[0:KS[g], g, j * CH:(j + 1) * CH])
                ins.then_inc(t_sem, 1)

        @block.vector
        def _(v):
            for g in range(NG):
                v.wait_ge(t_sem, g + 1)
                v.tensor_scalar_mul(csc[0:MS[g], g, :], pb[0:MS[g], g % 2, :],
                                    1.0).then_inc(cv_sem, 1)

    nc.compile()
    return nc


_CACHE = {}
_LAST_RESULT = None


def kernel(x, emb_table):
    global _LAST_RESULT
    from concourse.bass_utils import run_bass_kernel_spmd

    x_np = np.asarray(x)
    emb_np = np.asarray(emb_table)
    uniq, cores, key = _prepare(x_np)
    table16 = np.zeros((NV + 1, D), np.float16)
    if uniq is None:
        table16[:NV] = emb_np[:NV].astype(np.float16)
    else:
        table16[:len(uniq)] = emb_np[uniq].astype(np.float16)

    if key not in _CACHE:
        _CACHE[key] = _build_program(key)
    nc = _CACHE[key]

    in_maps = [{"table": table16, "idx": co["idx"], "tidx": co["tidx"],
                "dmat": co["dmat"]} for co in cores]
    res = run_bass_kernel_spmd(nc, in_maps, core_ids=list(range(N_CORES)))
    _LAST_RESULT = res
    full = np.empty((B, S, D), dtype=np.float16)
    for c in range(N_CORES):
        b, h = c // 2, c % 2
        full[b, h * RPC:(h + 1) * RPC, :] = res.results[c]["out"][:RPC]
    return full.astype(np.float32)


# revision 46
# speedup vs baseline: 1.0136x; 1.0136x over previous
"""Trainium2 Bass kernel for nn_BlankEmbedding (embedding gather + blank-run scan).

Math: the reference computes e = emb_table[x] then 8 shift/accumulate
iterations seeded at pre-blank positions.  Unrolled, out[i] =
sum_{d=0..8} C[i,d] * e[i-d] with banded integer coefficients C that
depend only on x.  Rows with any C[i,d>0] != 0 ("affected") are sparse
(~6% at the reference blank density); every other row is out[i] = e[i].

Kernel design (per core: 2048 of the 16384 [B*S] rows, data-parallel):
  - the table is cast to fp16 on host (values ~N(0, 0.02); fp16 rounding
    is ~5e-4 relative vs the 2e-2 gate), halving all device traffic; the
    host upcasts the assembled output back to fp32.
  - 16 aligned tiles of 128 rows: ONE hardware-indirect DMA gathers the
    tile's embedding rows ([128,1] int32 offset column read by the DGE -
    no per-row ucode), and a dense writeback DMA streams it straight
    back out SBUF -> DRAM.  This pure gather->writeback pipeline is
    DMA-bound; TensorE/DVE do nothing for the 94% identity rows.
  - affected rows are packed into NG compact groups (<=128 targets and
    <=128 deduped band-source rows each).  Per group: one indirect
    gather of the sources (issued FIRST so it never waits behind the
    main stream), 4 matmuls (fp16 x fp16 -> fp32 PSUM, 512-col chunks =
    one PSUM bank) with a host-built [src,tgt] coefficient matrix that
    includes the d=0 term, a DVE copy PSUM -> fp16 SBUF, and one
    indirect-scatter DMA that OVERWRITES the affected output rows after
    every direct writeback has completed.
  - padding: dead source slots point at an all-zero table row (row NV);
    dead target slots scatter into a dummy output row (row RPC).

Sync notes (hard-won): a DMA's +16 semaphore update arrives as 16
per-engine +1s, so every gather/writeback that gates something gets its
own semaphore; and engine program order does not cover a compute op's
SBUF write drain before a same-engine dma_start, so DMA-after-compute
is always gated through a semaphore (here: copies on DVE, scatters on
GPSIMD).

All differences between cores are input data (indices, coefficients),
so one program per group-count NG is compiled and reused.
"""

import numpy as np

B, S, D = 4, 4096, 2048
N_CORES = 8
RPC = (B * S) // N_CORES   # rows per core = 2048
NV = 2048                  # x < 2048 per the problem spec -> table slice
NTM = 16                   # main tiles of 128 rows
TSPLIT = 12                # scatter gate split: tiles [0,12) vs [12,16)
RSPLIT = TSPLIT * 128      # groups never span this target-row boundary
K = 128
NB = 9                     # band width: out[i] depends on e[i-8..i]
CH = 512                   # matmul N-chunk = one PSUM bank of fp32
NCH = D // CH
N_ITER = 8


def _compute_coeffs(x):
    """C[b, s, d] for d=0..8 (float64 holds small ints exactly)."""
    b, s = x.shape
    blank = ((x >= 0) & (x < 16)).astype(np.float64)
    shift_r = lambda t: np.concatenate([np.zeros_like(t[:, :1]), t[:, :-1]], axis=1)
    first = np.maximum(blank - shift_r(blank), 0.0)
    m = np.concatenate([first[:, 1:], np.zeros_like(first[:, :1])], axis=1)
    C = np.zeros((b, s, NB))
    C[:, :, 0] = 1.0
    for k in range(1, N_ITER + 1):
        m_k = np.zeros_like(m)
        m_k[:, k:] = m[:, :-k]
        Cs = np.zeros_like(C)
        Cs[:, 1:, 1:] = C[:, :-1, :-1]
        C = C + m_k[:, :, None] * Cs
    return C


def _prepare(x_np):
    """Per-core gather indices, scatter targets, group coefficient mats."""
    if x_np.max() < NV and x_np.min() >= 0:
        ridx = x_np.astype(np.int64)
        uniq = None
    else:  # fallback: remap through unique rows (stays within NV slots)
        uniq, inv = np.unique(x_np, return_inverse=True)
        assert len(uniq) <= NV
        ridx = inv.reshape(x_np.shape).astype(np.int64)

    C = _compute_coeffs(x_np)
    cores = []
    for c in range(N_CORES):
        b, h = c // 2, c % 2
        s0 = h * RPC
        # greedy pack affected targets into groups of <=128 targets with
        # <=128 distinct source positions; a group never spans the RSPLIT
        # boundary so each scatter can gate on only its half's writebacks
        # one early group (targets below RSPLIT, scatter gated on w_lo
        # only) + late groups (everything after its first overflow or the
        # boundary; gated on all writebacks).  A single spill group would
        # cost a fixed ~1.1us gather-issue slot mid-stream, so spills ride
        # with the late stream instead.
        glo, ghi = [], []     # group lists: (srcs, [(tgt, {d: coef})])
        cur_s, cur_t = [], []
        lo_open = True
        for r in range(RPC):
            if not (C[b, s0 + r, 1:] != 0).any():
                continue
            coefs = {int(d): C[b, s0 + r, d] for d in range(NB)
                     if C[b, s0 + r, d] != 0}
            srcs = [s0 + r - d for d in coefs]
            new = [p for p in srcs if p not in cur_s]
            overflow = len(cur_s) + len(new) > K or len(cur_t) + 1 > K
            if lo_open and (r >= RSPLIT or overflow):
                if cur_t:
                    glo.append((cur_s, cur_t))
                cur_s, cur_t = [], []
                lo_open = False
                new, overflow = srcs, False
            if overflow:
                ghi.append((cur_s, cur_t))
                cur_s, cur_t = list(dict.fromkeys(srcs)), [(r, coefs)]
            else:
                cur_s += new
                cur_t.append((r, coefs))
        if cur_t:
            (glo if lo_open else ghi).append((cur_s, cur_t))
        cores.append(dict(b=b, s0=s0, glo=glo, ghi=ghi))

    # uniform slot layout: region-0 group slots first, then region-1
    N0 = max(len(co["glo"]) for co in cores)
    N1 = max(len(co["ghi"]) for co in cores)
    NG = N0 + N1
    REGS = (0,) * N0 + (2,) * N1   # 2: may hold low-half spill -> gate all

    def slot_group(co, g):
        lst, gi = (co["glo"], g) if g < N0 else (co["ghi"], g - N0)
        return lst[gi] if gi < len(lst) else None

    # exact per-group capacities (max over cores; floor 2 because
    # single-element indirect DMAs are unsupported)
    KS, MS = [], []
    for g in range(NG):
        gs = [slot_group(co, g) for co in cores]
        KS.append(max(2, max(len(gr[0]) if gr else 1 for gr in gs)))
        MS.append(max(2, max(len(gr[1]) if gr else 1 for gr in gs)))
    for co in cores:
        b, s0 = co["b"], co["s0"]
        # one extra column: tile 15 is gathered/written as two 64-row
        # halves to halve the end-of-stream drain
        idx = np.zeros((K, NTM + NG + 1), np.int32)
        for t in range(NTM):
            idx[:, NG + t] = ridx[b, s0 + t * K: s0 + (t + 1) * K]
        idx[0:64, NG + NTM] = ridx[b, s0 + 15 * K + 64: s0 + 16 * K]
        tidx = np.full((K, NG), RPC, np.int32)        # pad -> dummy row
        dmat = np.zeros((K, NG * K), np.float16)
        for g in range(NG):
            idx[:, g] = NV                            # pad -> zero row
            gr = slot_group(co, g)
            if gr is not None:
                srcs, tgts = gr
                spos = {p: k for k, p in enumerate(srcs)}
                idx[:len(srcs), g] = [ridx[b, p] for p in srcs]
                for m_i, (r, coefs) in enumerate(tgts):
                    tidx[m_i, g] = r
                    for d, cf in coefs.items():
                        dmat[spos[s0 + r - d], g * K + m_i] = cf
        co.update(idx=idx, tidx=tidx, dmat=dmat)
    return uniq, cores, (NG, REGS, tuple(KS), tuple(MS))


def _build_program(key):
    import concourse.bacc as bacc
    import concourse.mybir as mybir
    from concourse.bass import IndirectOffsetOnAxis

    NG, REGS, KS, MS = key
    f16, f32, i32 = mybir.dt.float16, mybir.dt.float32, mybir.dt.int32
    NTI = NTM + NG + 1        # + tile-15 second half (drain split)
    NW_LO = TSPLIT            # writebacks gating region-0 scatters
    NW_HI = NTM - TSPLIT + 1  # high tiles incl. the tile-15 extra half
    nc = bacc.Bacc("TRN2", target_bir_lowering=False, debug=False,
                   enable_asserts=False, num_devices=N_CORES)
    table_d = nc.dram_tensor("table", [NV + 1, D], f16, kind="ExternalInput")
    idx_d = nc.dram_tensor("idx", [K, NTI], i32, kind="ExternalInput")
    tidx_d = nc.dram_tensor("tidx", [K, NG], i32, kind="ExternalInput")
    dmat_d = nc.dram_tensor("dmat", [K, NG * K], f16, kind="ExternalInput")
    out_d = nc.dram_tensor("out", [RPC + 1, D], f16, kind="ExternalOutput")

    from contextlib import ExitStack
    with ExitStack() as st:
        gtile = st.enter_context(nc.sbuf_tensor("gtile", [K, NTI, D], f16))
        csc = st.enter_context(nc.sbuf_tensor("csc", [K, NG, D], f16))
        dmat_s = st.enter_context(nc.sbuf_tensor("dmat_s", [K, NG * K], f16))
        idx_s = st.enter_context(nc.sbuf_tensor("idx_s", [K, NTI], i32))
        tidx_s = st.enter_context(nc.sbuf_tensor("tidx_s", [K, NG], i32))
        pb = st.enter_context(nc.psum_tensor("pb", [K, 2, D], f32))
        ix_sem = st.enter_context(nc.semaphore("ix_sem"))
        dm_sem = st.enter_context(nc.semaphore("dm_sem"))
        g_sems = [st.enter_context(nc.semaphore(f"g_sem{t}")) for t in range(NTI)]
        t_sem = st.enter_context(nc.semaphore("t_sem"))
        cv_sem = st.enter_context(nc.semaphore("cv_sem"))
        w_lo = st.enter_context(nc.semaphore("w_lo"))
        w_hi = st.enter_context(nc.semaphore("w_hi"))
        s_sem = st.enter_context(nc.semaphore("s_sem"))
        block = st.enter_context(nc.Block(no_gpsimd_drain=True))

        def writeback(eng, t):
            # tile 15 is written as two 64-row halves on DIFFERENT rings
            # (half A here on Act, half B on sync) so the stream drain runs
            # both HWDGE rings in parallel
            ws = w_lo if t < TSPLIT else w_hi
            n = 64 if t == NTM - 1 else K
            eng.wait_ge(g_sems[NG + t], 16)
            eng.dma_start(out_d[t * K:t * K + n, :],
                          gtile[0:n, NG + t, :]).then_inc(ws, 16)

        @block.sync
        def _(sy):
            sy.dma_start(idx_s[:, :], idx_d[:, :]).then_inc(ix_sem, 16)
            for t in range(0, NTM, 2):
                writeback(sy, t)
            sy.wait_ge(g_sems[NG + NTM], 16)   # tile-15 half B
            sy.dma_start(out_d[NTM * K - 64:NTM * K, :],
                         gtile[0:64, NG + NTM, :]).then_inc(w_hi, 16)
            sy.wait_ge(w_lo, 16 * NW_LO)
            sy.wait_ge(w_hi, 16 * NW_HI)
            sy.wait_ge(s_sem, 16 * NG)

        @block.scalar
        def _(sc):
            sc.dma_start(dmat_s[:, :], dmat_d[:, :]).then_inc(dm_sem, 16)
            sc.dma_start(tidx_s[:, :], tidx_d[:, :]).then_inc(dm_sem, 16)
            for t in range(1, NTM, 2):
                writeback(sc, t)

        # first two tile gathers lead (earliest writeback start), then the
        # group sources, then the rest; tile 15's halves close the stream
        issue = ([NG, NG + 1] + list(range(NG))
                 + list(range(NG + 2, NG + NTM)) + [NG + NTM])

        @block.gpsimd
        def _(gp):
            gp.wait_ge(ix_sem, 16)
            for t in issue:
                if t < NG:
                    kk = KS[t]
                elif t == NG + NTM - 1 or t == NG + NTM:
                    kk = 64               # tile-15 halves
                else:
                    kk = K
                gp.indirect_dma_start(
                    out=gtile[0:kk, t, :], out_offset=None,
                    in_=table_d[:, :],
                    in_offset=IndirectOffsetOnAxis(ap=idx_s[0:kk, t:t + 1], axis=0),
                ).then_inc(g_sems[t], 16)
            gp.wait_ge(dm_sem, 32)         # tidx loaded
            for g in range(NG):
                # only tiles holding this group's targets must be written;
                # early-group scatters fire while the high tiles still stream
                gp.wait_ge(w_lo, 16 * NW_LO)
                if REGS[g] != 0:
                    gp.wait_ge(w_hi, 16 * NW_HI)
                gp.wait_ge(cv_sem, g + 1)  # corrected rows staged in csc
                gp.indirect_dma_start(
                    out=out_d[:, :],
                    out_offset=IndirectOffsetOnAxis(ap=tidx_s[0:MS[g], g:g + 1],
                                                    axis=0),
                    in_=csc[0:MS[g], g, :], in_offset=None,
                ).then_inc(s_sem, 16)

        @block.tensor
        def _(te):
            te.wait_ge(dm_sem, 32)
            for g in range(NG):
                te.wait_ge(g_sems[g], 16)
                if g >= 2:
                    te.wait_ge(cv_sem, g - 1)  # PSUM slot g%2 free again
                for j in range(NCH):
                    ins = te.matmul(pb[0:MS[g], g % 2, j * CH:(j + 1) * CH],
                                    dmat_s[0:KS[g], g * K:g * K + MS[g]],
                                    gtile[0:KS[g], g, j * CH:(j + 1) * CH])
                ins.then_inc(t_sem, 1)

        @block.vector
        def _(v):
            for g in range(NG):
                v.wait_ge(t_sem, g + 1)
                v.tensor_scalar_mul(csc[0:MS[g], g, :], pb[0:MS[g], g % 2, :],
                                    1.0).then_inc(cv_sem, 1)

    nc.compile()
    return nc


_CACHE = {}
_LAST_RESULT = None


def kernel(x, emb_table):
    global _LAST_RESULT
    from concourse.bass_utils import run_bass_kernel_spmd

    x_np = np.asarray(x)
    emb_np = np.asarray(emb_table)
    uniq, cores, key = _prepare(x_np)
    table16 = np.zeros((NV + 1, D), np.float16)
    if uniq is None:
        table16[:NV] = emb_np[:NV].astype(np.float16)
    else:
        table16[:len(uniq)] = emb_np[uniq].astype(np.float16)

    if key not in _CACHE:
        _CACHE[key] = _build_program(key)
    nc = _CACHE[key]

    in_maps = [{"table": table16, "idx": co["idx"], "tidx": co["tidx"],
                "dmat": co["dmat"]} for co in cores]
    res = run_bass_kernel_spmd(nc, in_maps, core_ids=list(range(N_CORES)))
    _LAST_RESULT = res
    full = np.empty((B, S, D), dtype=np.float16)
    for c in range(N_CORES):
        b, h = c // 2, c % 2
        full[b, h * RPC:(h + 1) * RPC, :] = res.results[c]["out"][:RPC]
    return full.astype(np.float32)
